# revision 16
# baseline (speedup 1.0000x reference)
"""BDC (Brownian Distance Covariance) pooling kernel for 8x Trainium2 cores.

kernel(x, t) -> [64, 205120] float32
  x: [64, 640, 100] float32, t: [1, 1] float32

Pure batch data-parallelism, 8 batches per NeuronCore. Per batch:
  - z_ij = d_i + d_j - 2 x_i.x_j via one augmented K=104 TensorE matmul in
    float32r (13-bit mantissa, 1 cyc/row at N>=256). A = [xT; d_hi; d_lo;
    1; 1], B = [-2 xT; 1; 1; d_hi; d_lo]; the hi/lo split keeps the d
    terms exact so the near-zero diagonal stays above -eps under sqrt.
  - d_i from a fused ScalarE square+row-accumulate over the x tiles.
  - dcov = sqrt(exp(t) * z + eps) on ScalarE with fused row sums; the
    dcov diagonal is then repaired to sqrt(eps) with a GpSimd
    affine_select (the reference clips z at 0; true diag z is 0).
  - double centering via rm'_i = rm_i - gm/2 (grand mean folded into the
    row means; cross-partition total via a ones-matmul) and a K=2 float32r
    matmul M2 = rm'_i + rm'_j, subtracted by one VectorE tensor_tensor per
    upper-trapezoid tile.
  - upper trapezoid (block r keeps cols >= 128 r) DMA'd out; the host maps
    trapezoid -> packed upper-triangle rows.
All matmul moving dims are kept >= 256 (junk-padded to 768 where needed)
to stay in float32r's 1 cycle/row regime. A/Bm/L2/R2 are manually
double-buffered persistent tiles so constant rows are written once.
"""
import os
from contextlib import ExitStack

import numpy as np

import concourse.bass as bass
import concourse.tile as tile
from concourse import bacc, mybir
from concourse.bass_utils import run_bass_kernel_spmd
from concourse.masks import make_identity

P = 128
M = 100
DIM = 640
DPAD = 768
NR = DIM // P
EPS = 1e-5
F32 = mybir.dt.float32
F32R = mybir.dt.float32r
NCORES = 8
BPC = 8

TRAP_W = [DIM - P * r for r in range(NR)]
TRAP_OFF = [P * sum(TRAP_W[:r]) for r in range(NR)]
TRAP_SIZE = P * sum(TRAP_W)

_USE_SCALAR_DMA = os.environ.get("BDC_SCALAR_DMA", "1") == "1"
_USE_GPSIMD_DMA = os.environ.get("BDC_GPSIMD_DMA", "1") == "1"


def _dma_eng(nc):
    return nc.scalar if _USE_SCALAR_DMA else nc.sync


LAST_EXEC_NS = None
LAST_RESULT = None
_PROGRAM = None


def _build(nc, n_batch):
    x_dram = nc.dram_tensor("x", [n_batch, DIM, M], F32, kind="ExternalInput").ap()
    t_dram = nc.dram_tensor("t", [1, 1], F32, kind="ExternalInput").ap()
    out_dram = nc.dram_tensor("out", [n_batch, TRAP_SIZE], F32,
                              kind="ExternalOutput").ap()

    with tile.TileContext(nc) as tc, ExitStack() as ctx:
        const = ctx.enter_context(tc.tile_pool(name="const", bufs=1))
        sb = ctx.enter_context(tc.tile_pool(name="sb", bufs=2))
        dcv = ctx.enter_context(tc.tile_pool(name="dcv", bufs=2 * NR))
        ps_z = ctx.enter_context(tc.tile_pool(name="ps_z", bufs=2, space="PSUM"))
        ps_tp = ctx.enter_context(tc.tile_pool(name="ps_tp", bufs=1, space="PSUM"))
        ps_m2 = ctx.enter_context(tc.tile_pool(name="ps_m2", bufs=1, space="PSUM"))

        # ---- constants ----
        ident_f = const.tile([P, P], F32)
        make_identity(nc, ident_f[:])
        ident_rt = const.tile([P, P], F32R)
        nc.vector.tensor_scalar_mul(ident_rt[:], ident_f[:], 1.0)
        ident_r = ident_rt[:]
        ones2_f = const.tile([2, DPAD], F32)
        nc.vector.memset(ones2_f[:], 1.0)
        zeros_f = const.tile([P, P], F32)
        nc.vector.memset(zeros_f[:], 0.0)
        ones128 = const.tile([P, P], F32)
        nc.vector.memset(ones128[:], 1.0)
        ones2_r = const.tile([2, DPAD], F32R)
        nc.vector.tensor_scalar_mul(ones2_r[:], ones2_f[:], 1.0)
        eps_vec = const.tile([P, 1], F32)
        nc.vector.memset(eps_vec[:], EPS)
        t_vec = const.tile([P, 1], F32)
        nc.sync.dma_start(out=t_vec[:], in_=t_dram[0:1, 0:1].to_broadcast((P, 1)))
        s_vec = const.tile([P, 1], F32)
        nc.scalar.activation(s_vec[:], t_vec[:], mybir.ActivationFunctionType.Exp)

        # manually double-buffered operand tiles; constant rows written once
        NBUF = n_batch
        A_t, Bm_t, L2_t, R2_t = [], [], [], []
        for i in range(NBUF):
            A = const.tile([104, DIM], F32R)
            nc.sync.dma_start(out=A[102:104, :], in_=ones2_r[:, 0:DIM])
            A_t.append(A)
            Bm = const.tile([104, DPAD], F32R)
            nc.vector.tensor_scalar_mul(Bm[0:104, DIM:DPAD], zeros_f[0:104, :], 1.0)
            nc.sync.dma_start(out=Bm[100:102, 0:DIM], in_=ones2_r[:, 0:DIM])
            Bm_t.append(Bm)
            L2 = const.tile([4, DPAD], F32R)
            nc.vector.tensor_scalar_mul(L2[0:2, :], ones2_f[0:2, :], 1.0)
            L2_t.append(L2)
            R2 = const.tile([4, DPAD], F32R)
            nc.vector.tensor_scalar_mul(R2[0:2, :], ones2_f[0:2, :], 0.0)
            nc.sync.dma_start(out=R2[2:4, :], in_=ones2_r[0:2, :])
            R2_t.append(R2)

        sqrt_eps = float(EPS ** 0.5)

        for b in range(n_batch):
            A = A_t[b % NBUF]
            Bm = Bm_t[b % NBUF]
            L2 = L2_t[b % NBUF]
            R2 = R2_t[b % NBUF]

            # ---- load + round x ----
            x_in = sb.tile([P, NR, M], F32, tag="x_in")
            nc.scalar.dma_start(out=x_in[:],
                              in_=x_dram[b].rearrange("(r p) m -> p r m", p=P))
            x_r = sb.tile([P, NR, M], F32R, tag="x_r")
            nc.vector.tensor_scalar_mul(x_r[:], x_in[:], 1.0)

            # ---- A rows 0..99 = x~T (5 PE transposes into one psum tile) ----
            tp = ps_tp.tile([M, DIM], F32R, tag="tp")
            for r in range(NR):
                nc.tensor.transpose(tp[:, r * P:(r + 1) * P], x_r[:, r, :], ident_r)
            nc.scalar.copy(A[0:M, :], tp[:])

            # ---- d_i = sum_m x~^2 via fused square+accum on x-layout ----
            db5 = sb.tile([P, NR], F32, tag="db5")
            for r in range(NR):
                sqs = sb.tile([P, M], F32, tag="sqs")
                nc.scalar.activation(sqs[:], x_r[:, r, :].bitcast(F32),
                                     mybir.ActivationFunctionType.Square,
                                     accum_out=db5[:, r:r + 1])
            # hi/lo split: dpack = [d_hi (cols 0:5) | d_lo (cols 5:10)]
            dpack = sb.tile([P, 2 * NR], F32R, tag="dpack")
            nc.vector.tensor_scalar_mul(dpack[:, 0:NR], db5[:], 1.0)
            nc.vector.tensor_tensor(dpack[:, NR:2 * NR], db5[:],
                                    dpack[:, 0:NR].bitcast(F32),
                                    mybir.AluOpType.subtract)
            tdp = ps_tp.tile([2 * NR, P], F32R, tag="tp")
            nc.tensor.transpose(tdp[:], dpack[:], ident_r)
            Td10 = sb.tile([2 * NR, P], F32R, tag="Td10")
            nc.scalar.copy(Td10[:], tdp[:])

            # ---- aug rows (partition 100+) via SBUF->SBUF DMAs ----
            _dma_eng(nc).dma_start(out=A[100:102, :], in_=Td10[:])
            _dma_eng(nc).dma_start(out=Bm[102:104, 0:DIM], in_=Td10[:])

            nc.vector.tensor_scalar_mul(Bm[0:M, 0:DIM], A[0:M, :].bitcast(F32), -2.0)

            # ---- z matmuls (K=104, fp32r) + sqrt with fused row sums ----
            rs5 = sb.tile([P, NR], F32, tag="rs5")
            dcov = []
            for r in range(NR):
                z_ps = ps_z.tile([P, DPAD], F32, tag="z")
                nc.tensor.matmul(z_ps[:, 0:512], A[:, r * P:(r + 1) * P],
                                 Bm[:, 0:512], start=True, stop=True)
                nc.tensor.matmul(z_ps[:, 512:DPAD], A[:, r * P:(r + 1) * P],
                                 Bm[:, 512:DPAD], start=True, stop=True)
                dc = dcv.tile([P, DIM], F32, tag="dcov")
                nc.scalar.activation(dc[:], z_ps[:, 0:DIM],
                                     mybir.ActivationFunctionType.Sqrt,
                                     bias=eps_vec[:], scale=s_vec[:],
                                     accum_out=rs5[:, r:r + 1])
                # repair the diagonal block: true diag of z is exactly 0
                nc.gpsimd.affine_select(
                    out=dc[:, r * P:(r + 1) * P],
                    in_=dc[:, r * P:(r + 1) * P],
                    compare_op=mybir.AluOpType.not_equal,
                    fill=sqrt_eps,
                    base=0,
                    pattern=[[-1, P]],
                    channel_multiplier=1,
                )
                dcov.append(dc)

            # ---- rm' = rs/640 - gm/2 (grand mean via ones-matmul) ----
            rowsum5 = sb.tile([P, 1], F32, tag="rowsum5")
            nc.vector.tensor_reduce(out=rowsum5[:], in_=rs5[:],
                                    axis=mybir.AxisListType.X,
                                    op=mybir.AluOpType.add)
            gm_ps = ps_tp.tile([P, 1], F32, tag="tp")
            nc.tensor.matmul(gm_ps[:], ones128[:], rowsum5[:], start=True, stop=True)
            gmh = sb.tile([P, 1], F32, tag="gmh")
            nc.vector.tensor_scalar_mul(gmh[:], gm_ps[:], 0.5 / (DIM * DIM))
            rm5 = sb.tile([P, NR], F32, tag="rm5")
            nc.vector.tensor_scalar(rm5[:], rs5[:], 1.0 / DIM, gmh[:],
                                    mybir.AluOpType.mult,
                                    mybir.AluOpType.subtract)
            rmpack = sb.tile([P, 2 * NR], F32R, tag="rmpack")
            nc.vector.tensor_scalar_mul(rmpack[:, 0:NR], rm5[:], 1.0)
            nc.vector.tensor_tensor(rmpack[:, NR:2 * NR], rm5[:],
                                    rmpack[:, 0:NR].bitcast(F32),
                                    mybir.AluOpType.subtract)
            trm = ps_tp.tile([2 * NR, P], F32R, tag="tp")
            nc.tensor.transpose(trm[:], rmpack[:], ident_r)
            T10 = sb.tile([2 * NR, P], F32R, tag="T10")
            nc.scalar.copy(T10[:], trm[:])
            _dma_eng(nc).dma_start(out=L2[2:4, 0:DIM], in_=T10[:])
            _dma_eng(nc).dma_start(out=R2[0:2, 0:DIM], in_=T10[:])

            # ---- centering (M2 = rm'_i + rm'_j via K=2 fp32r matmul) ----
            for r in range(NR):
                w = TRAP_W[r]
                c0 = r * P
                m2 = ps_m2.tile([P, DPAD], F32, tag="m2")
                n0 = 0
                while n0 < w:
                    nn = min(512, w - n0)
                    if nn < 256:
                        nn = min(256, DPAD - c0 - n0)  # junk-pad to >=256
                    nc.tensor.matmul(m2[:, n0:n0 + nn], L2[0:4, c0:c0 + P],
                                     R2[:, c0 + n0:c0 + n0 + nn],
                                     start=True, stop=True)
                    n0 += nn
                nc.vector.tensor_tensor(dcov[r][:, c0:DIM], dcov[r][:, c0:DIM],
                                        m2[:, 0:w], mybir.AluOpType.subtract)
                eng = nc.sync
                eng.dma_start(
                    out=out_dram[b, TRAP_OFF[r]:TRAP_OFF[r] + P * w]
                        .rearrange("(p w) -> p w", p=P),
                    in_=dcov[r][:, c0:DIM],
                )
    return nc


def _get_program():
    global _PROGRAM
    if _PROGRAM is None:
        nc = bacc.Bacc("TRN2", target_bir_lowering=False, debug=False)
        _build(nc, BPC)
        nc.compile()
        _PROGRAM = nc
    return _PROGRAM


def _triu_index_map():
    iu_r, iu_c = np.triu_indices(DIM)
    r = iu_r // P
    off = np.array(TRAP_OFF)[r]
    w = np.array(TRAP_W)[r]
    return (off + (iu_r - r * P) * w + (iu_c - r * P)).astype(np.int64)


def kernel(x, t):
    global LAST_EXEC_NS, LAST_RESULT
    x = np.ascontiguousarray(np.asarray(x, dtype=np.float32))
    t = np.ascontiguousarray(np.asarray(t, dtype=np.float32)).reshape(1, 1)
    B = x.shape[0]
    assert x.shape == (B, DIM, M) and B == NCORES * BPC

    nc = _get_program()
    in_maps = [{"x": x[c * BPC:(c + 1) * BPC], "t": t} for c in range(NCORES)]
    trace = os.environ.get("BDC_TRACE", "0") == "1"
    res = run_bass_kernel_spmd(nc, in_maps, list(range(NCORES)), trace=trace)
    LAST_EXEC_NS = res.exec_time_ns
    LAST_RESULT = res

    trap = np.concatenate([res.results[c]["out"] for c in range(NCORES)], axis=0)
    return trap[:, _triu_index_map()]


# revision 17
# speedup vs baseline: 1.1081x; 1.1081x over previous
"""BDC (Brownian Distance Covariance) pooling kernel for 8x Trainium2 cores.

kernel(x, t) -> [64, 205120] float32
  x: [64, 640, 100] float32, t: [1, 1] float32

Pure batch data-parallelism, 8 batches per NeuronCore. Per batch:
  - z_ij = d_i + d_j - 2 x_i.x_j via one augmented K=104 TensorE matmul in
    float32r (13-bit mantissa, 1 cyc/row at N>=256). A = [xT; d_hi; d_lo;
    1; 1], B = [-2 xT; 1; 1; d_hi; d_lo]; the hi/lo split keeps the d
    terms exact so the near-zero diagonal stays above -eps under sqrt.
  - d_i from a fused ScalarE square+row-accumulate over the x tiles.
  - dcov = sqrt(exp(t) * z + eps) on ScalarE with fused row sums; the
    dcov diagonal is then repaired to sqrt(eps) with a GpSimd
    affine_select (the reference clips z at 0; true diag z is 0).
  - double centering via rm'_i = rm_i - gm/2 (grand mean folded into the
    row means; cross-partition total via a ones-matmul) and a K=2 float32r
    matmul M2 = rm'_i + rm'_j, subtracted by one VectorE tensor_tensor per
    upper-trapezoid tile.
  - upper trapezoid (block r keeps cols >= 128 r) DMA'd out; the host maps
    trapezoid -> packed upper-triangle rows.
All matmul moving dims are kept >= 256 (junk-padded to 768 where needed)
to stay in float32r's 1 cycle/row regime. A/Bm/L2/R2 are manually
double-buffered persistent tiles so constant rows are written once.
"""
import os
from contextlib import ExitStack

import numpy as np

import concourse.bass as bass
import concourse.tile as tile
from concourse import bacc, mybir
from concourse.bass_utils import run_bass_kernel_spmd
from concourse.masks import make_identity

P = 128
M = 100
DIM = 640
DPAD = 768
NR = DIM // P
EPS = 1e-5
F32 = mybir.dt.float32
F32R = mybir.dt.float32r
NCORES = 8
BPC = 8

TRAP_W = [DIM - P * r for r in range(NR)]
TRAP_OFF = [P * sum(TRAP_W[:r]) for r in range(NR)]
TRAP_SIZE = P * sum(TRAP_W)

_USE_SCALAR_DMA = os.environ.get("BDC_SCALAR_DMA", "1") == "1"
_USE_GPSIMD_DMA = os.environ.get("BDC_GPSIMD_DMA", "1") == "1"


def _dma_eng(nc):
    return nc.scalar if _USE_SCALAR_DMA else nc.sync


LAST_EXEC_NS = None
LAST_RESULT = None
_PROGRAM = None


def _build(nc, n_batch):
    x_dram = nc.dram_tensor("x", [n_batch, DIM, M], F32, kind="ExternalInput").ap()
    t_dram = nc.dram_tensor("t", [1, 1], F32, kind="ExternalInput").ap()
    out_dram = nc.dram_tensor("out", [n_batch, TRAP_SIZE], F32,
                              kind="ExternalOutput").ap()

    with tile.TileContext(nc) as tc, ExitStack() as ctx:
        const = ctx.enter_context(tc.tile_pool(name="const", bufs=1))
        sb = ctx.enter_context(tc.tile_pool(name="sb", bufs=2))
        dcv = ctx.enter_context(tc.tile_pool(name="dcv", bufs=2 * NR))
        ps_z = ctx.enter_context(tc.tile_pool(name="ps_z", bufs=2, space="PSUM"))
        ps_tp = ctx.enter_context(tc.tile_pool(name="ps_tp", bufs=1, space="PSUM"))
        ps_m2 = ctx.enter_context(tc.tile_pool(name="ps_m2", bufs=1, space="PSUM"))
        ps_zs = ps_z

        # ---- constants ----
        ident_f = const.tile([P, P], F32)
        make_identity(nc, ident_f[:])
        ident_rt = const.tile([P, P], F32R)
        nc.vector.tensor_scalar_mul(ident_rt[:], ident_f[:], 1.0)
        ident_r = ident_rt[:]
        ones2_f = const.tile([2, DPAD], F32)
        nc.vector.memset(ones2_f[:], 1.0)
        zeros_f = const.tile([P, P], F32)
        nc.vector.memset(zeros_f[:], 0.0)
        ones128 = const.tile([P, P], F32)
        nc.vector.memset(ones128[:], 1.0)
        ones2_r = const.tile([2, DPAD], F32R)
        nc.vector.tensor_scalar_mul(ones2_r[:], ones2_f[:], 1.0)
        eps_vec = const.tile([P, 1], F32)
        nc.vector.memset(eps_vec[:], EPS)
        t_vec = const.tile([P, 1], F32)
        nc.sync.dma_start(out=t_vec[:], in_=t_dram[0:1, 0:1].to_broadcast((P, 1)))
        s_vec = const.tile([P, 1], F32)
        nc.scalar.activation(s_vec[:], t_vec[:], mybir.ActivationFunctionType.Exp)

        # manually double-buffered operand tiles; constant rows written once
        NBUF = n_batch
        A_t, Bm_t, L2_t, R2_t = [], [], [], []
        for i in range(NBUF):
            A = const.tile([104, DIM], F32R)
            nc.sync.dma_start(out=A[102:104, :], in_=ones2_r[:, 0:DIM])
            A_t.append(A)
            Bm = const.tile([104, DPAD], F32R)
            nc.vector.tensor_scalar_mul(Bm[0:104, DIM:DPAD], zeros_f[0:104, :], 1.0)
            nc.sync.dma_start(out=Bm[100:102, 0:DIM], in_=ones2_r[:, 0:DIM])
            Bm_t.append(Bm)
            L2 = const.tile([4, DPAD], F32R)
            nc.vector.tensor_scalar_mul(L2[0:2, :], ones2_f[0:2, :], 1.0)
            L2_t.append(L2)
            R2 = const.tile([4, DPAD], F32R)
            nc.vector.tensor_scalar_mul(R2[0:2, :], ones2_f[0:2, :], 0.0)
            nc.sync.dma_start(out=R2[2:4, :], in_=ones2_r[0:2, :])
            R2_t.append(R2)

        sqrt_eps = float(EPS ** 0.5)

        # hoist all input loads to the start (own slots, never contended)
        x_ins = []
        for b in range(n_batch):
            x_in = const.tile([P, NR, M], F32)
            nc.sync.dma_start(out=x_in[:],
                              in_=x_dram[b].rearrange("(r p) m -> p r m", p=P))
            x_ins.append(x_in)

        for b in range(n_batch):
            A = A_t[b % NBUF]
            Bm = Bm_t[b % NBUF]
            L2 = L2_t[b % NBUF]
            R2 = R2_t[b % NBUF]

            # ---- round x ----
            x_in = x_ins[b]
            x_r = sb.tile([P, NR, M], F32R, tag="x_r")
            nc.vector.tensor_scalar_mul(x_r[:], x_in[:], 1.0)

            # ---- A rows 0..99 = x~T (5 PE transposes into one psum tile) ----
            tp = ps_tp.tile([M, DIM], F32R, tag="tp")
            for r in range(NR):
                nc.tensor.transpose(tp[:, r * P:(r + 1) * P], x_r[:, r, :], ident_r)
            nc.scalar.copy(A[0:M, :], tp[:])

            # ---- d_i = sum_m x~^2 via fused square+accum on x-layout ----
            db5 = sb.tile([P, NR], F32, tag="db5")
            for r in range(NR):
                sqs = sb.tile([P, M], F32, tag="sqs")
                nc.scalar.activation(sqs[:], x_r[:, r, :].bitcast(F32),
                                     mybir.ActivationFunctionType.Square,
                                     accum_out=db5[:, r:r + 1])
            # hi/lo split: dpack = [d_hi (cols 0:5) | d_lo (cols 5:10)]
            dpack = sb.tile([P, 2 * NR], F32R, tag="dpack")
            nc.vector.tensor_scalar_mul(dpack[:, 0:NR], db5[:], 1.0)
            nc.vector.tensor_tensor(dpack[:, NR:2 * NR], db5[:],
                                    dpack[:, 0:NR].bitcast(F32),
                                    mybir.AluOpType.subtract)
            tdp = ps_zs.tile([2 * NR, P], F32R, tag="z")
            nc.tensor.transpose(tdp[:], dpack[:], ident_r)
            Td10 = sb.tile([2 * NR, P], F32R, tag="Td10")
            nc.scalar.copy(Td10[:], tdp[:])

            # ---- aug rows (partition 100+) via SBUF->SBUF DMAs ----
            _dma_eng(nc).dma_start(out=A[100:102, :], in_=Td10[:])
            _dma_eng(nc).dma_start(out=Bm[102:104, 0:DIM], in_=Td10[:])

            nc.vector.tensor_scalar_mul(Bm[0:M, 0:DIM], A[0:M, :].bitcast(F32), -2.0)

            # ---- z matmuls (K=104, fp32r) + sqrt with fused row sums ----
            rs5 = sb.tile([P, NR], F32, tag="rs5")
            dcov = []
            for r in range(NR):
                z_ps = ps_z.tile([P, DPAD], F32, tag="z")
                nc.tensor.matmul(z_ps[:, 0:512], A[:, r * P:(r + 1) * P],
                                 Bm[:, 0:512], start=True, stop=True)
                nc.tensor.matmul(z_ps[:, 512:DPAD], A[:, r * P:(r + 1) * P],
                                 Bm[:, 512:DPAD], start=True, stop=True)
                dc = dcv.tile([P, DIM], F32, tag="dcov")
                nc.scalar.activation(dc[:], z_ps[:, 0:DIM],
                                     mybir.ActivationFunctionType.Sqrt,
                                     bias=eps_vec[:], scale=s_vec[:],
                                     accum_out=rs5[:, r:r + 1])
                # repair the diagonal block: true diag of z is exactly 0
                nc.gpsimd.affine_select(
                    out=dc[:, r * P:(r + 1) * P],
                    in_=dc[:, r * P:(r + 1) * P],
                    compare_op=mybir.AluOpType.not_equal,
                    fill=sqrt_eps,
                    base=0,
                    pattern=[[-1, P]],
                    channel_multiplier=1,
                )
                dcov.append(dc)

            # ---- rm' = rs/640 - gm/2 (grand mean via ones-matmul) ----
            rowsum5 = sb.tile([P, 1], F32, tag="rowsum5")
            nc.vector.tensor_reduce(out=rowsum5[:], in_=rs5[:],
                                    axis=mybir.AxisListType.X,
                                    op=mybir.AluOpType.add)
            gm_ps = ps_zs.tile([P, 1], F32, tag="z")
            nc.tensor.matmul(gm_ps[:], ones128[:], rowsum5[:], start=True, stop=True)
            gmh = sb.tile([P, 1], F32, tag="gmh")
            nc.vector.tensor_scalar_mul(gmh[:], gm_ps[:], 0.5 / (DIM * DIM))
            rm5 = sb.tile([P, NR], F32, tag="rm5")
            nc.vector.tensor_scalar(rm5[:], rs5[:], 1.0 / DIM, gmh[:],
                                    mybir.AluOpType.mult,
                                    mybir.AluOpType.subtract)
            rmpack = sb.tile([P, 2 * NR], F32R, tag="rmpack")
            nc.vector.tensor_scalar_mul(rmpack[:, 0:NR], rm5[:], 1.0)
            nc.vector.tensor_tensor(rmpack[:, NR:2 * NR], rm5[:],
                                    rmpack[:, 0:NR].bitcast(F32),
                                    mybir.AluOpType.subtract)
            trm = ps_zs.tile([2 * NR, P], F32R, tag="z")
            nc.tensor.transpose(trm[:], rmpack[:], ident_r)
            T10 = sb.tile([2 * NR, P], F32R, tag="T10")
            nc.scalar.copy(T10[:], trm[:])
            _dma_eng(nc).dma_start(out=L2[2:4, 0:DIM], in_=T10[:])
            _dma_eng(nc).dma_start(out=R2[0:2, 0:DIM], in_=T10[:])

            # ---- centering (M2 = rm'_i + rm'_j via K=2 fp32r matmul) ----
            for r in range(NR):
                w = TRAP_W[r]
                c0 = r * P
                m2 = ps_m2.tile([P, DPAD], F32, tag="m2")
                n0 = 0
                while n0 < w:
                    nn = min(512, w - n0)
                    if nn < 256:
                        nn = min(256, DPAD - c0 - n0)  # junk-pad to >=256
                    nc.tensor.matmul(m2[:, n0:n0 + nn], L2[0:4, c0:c0 + P],
                                     R2[:, c0 + n0:c0 + n0 + nn],
                                     start=True, stop=True)
                    n0 += nn
                nc.vector.tensor_tensor(dcov[r][:, c0:DIM], dcov[r][:, c0:DIM],
                                        m2[:, 0:w], mybir.AluOpType.subtract)
                eng = nc.sync
                eng.dma_start(
                    out=out_dram[b, TRAP_OFF[r]:TRAP_OFF[r] + P * w]
                        .rearrange("(p w) -> p w", p=P),
                    in_=dcov[r][:, c0:DIM],
                )
    return nc


def _get_program():
    global _PROGRAM
    if _PROGRAM is None:
        nc = bacc.Bacc("TRN2", target_bir_lowering=False, debug=False)
        _build(nc, BPC)
        nc.compile()
        _PROGRAM = nc
    return _PROGRAM


def _triu_index_map():
    iu_r, iu_c = np.triu_indices(DIM)
    r = iu_r // P
    off = np.array(TRAP_OFF)[r]
    w = np.array(TRAP_W)[r]
    return (off + (iu_r - r * P) * w + (iu_c - r * P)).astype(np.int64)


def kernel(x, t):
    global LAST_EXEC_NS, LAST_RESULT
    x = np.ascontiguousarray(np.asarray(x, dtype=np.float32))
    t = np.ascontiguousarray(np.asarray(t, dtype=np.float32)).reshape(1, 1)
    B = x.shape[0]
    assert x.shape == (B, DIM, M) and B == NCORES * BPC

    nc = _get_program()
    in_maps = [{"x": x[c * BPC:(c + 1) * BPC], "t": t} for c in range(NCORES)]
    trace = os.environ.get("BDC_TRACE", "0") == "1"
    res = run_bass_kernel_spmd(nc, in_maps, list(range(NCORES)), trace=trace)
    LAST_EXEC_NS = res.exec_time_ns
    LAST_RESULT = res

    trap = np.concatenate([res.results[c]["out"] for c in range(NCORES)], axis=0)
    return trap[:, _triu_index_map()]
